# revision 11
# baseline (speedup 1.0000x reference)
"""Single-head attention (B=4, S=4096, E=2048, d=128) on 8 trn2 cores.

Sharding: core c handles (batch b = c//2, seq half h = c%2). Each core
projects q/k/v for its own 2048-row half; the pair (2b, 2b+1) exchanges
K and V halves via four pairwise AllGathers. Measured CC behavior: the
first mesh cannot begin before ~52us regardless of trigger time (NRT
arming), then meshes run serially at ~7-8us per 256KB — so a dummy
warmup AllGather is fired at ~8us to absorb the arming latency, and the
attention pass is split own/peer (8/8 k-pairs) so peer data is first
consumed ~95us, far behind the worst-case exchange completion (~85us).

Engine/queue plan (all measured):
  sync HWDGE ring: w pieces 0/1, x even e-chunks (both quarters),
    output stores. scalar HWDGE ring: bias, w pieces 2-4, x odd
    e-chunks. Two rings share ~360GB/s; a third (gpsimd SWDGE) path
    starves the scalar ring, so x stays on two rings.
  ACT queue: exp only (plus a tiny warmup activation to preload the
    Exp table before the pass). Projection evacuations run on the DVE
    (tensor_scalar_add with the [128,1] bias column) — ACT evacuations
    behind ring-credit-gated DMA issues measurably slip by >10us.
  gpsimd queue: collective staging DMAs in AND out + triggers. (cc_out
    landings on the sync ring got statically scheduled after pass
    stores, stalling pass-Y PV by ~4us.)

Projection: per streamed x chunk, K, V, Q matmuls (6 x N=512) per
quarter — PE-bound at ~1.28us/chunk vs ~0.72us arrival. V-half
transposes ([d,k]->[k,d] PE identity transposes): quarter-0's ride the
quarter-1 matmul stream, quarter-1's ride the first pass stage.
PSUM: ps_big 3x[128,1024] holds K,V,Q of one quarter, rotating into
the next quarter then score tiles; ps_acc/ps_small hold transposes
during projection, ps_o/ps_sum during the passes.

Attention: blocks of 8 k-pairs x 512 queries, software-pipelined:
block n's score matmuls interleave with block n-1's PV matmuls in the
PE FIFO, so the exp engines (ACT 6 + DVE-Schraudolph 2 per block, ~3%
max rel err, numerically validated to 1/2 of tiles) stay under the
~7.9us PE stage time and score PSUM tiles recycle with slack.
Denominators: DVE pair-add subtrees (leaf=4) at leaf boundaries +
exact ones-column matmuls after the PVs. Output/sums evacuate on the
DVE; host divides and transposes.
"""

import numpy as np
import ml_dtypes

import concourse.tile as tile
from concourse import bacc, mybir
from concourse.bass_utils import run_bass_kernel_spmd

N_CORES = 8
B, S, E, D = 4, 4096, 2048, 128
HALF = S // 2  # queries / own keys per core
QB = 512  # query block (PSUM bank width in fp32)
SQ = 1024  # projection quarter width
SCALE = 1.0 / float(np.sqrt(D))

BF16 = mybir.dt.bfloat16
F32 = mybir.dt.float32
AF = mybir.ActivationFunctionType

_CACHE = {}


def _build():
    nc = bacc.Bacc(
        trn_type="TRN2", target_bir_lowering=False, debug=False, num_devices=N_CORES
    )

    x_d = nc.dram_tensor("xt", [E, HALF], BF16, kind="ExternalInput").ap()
    w_d = nc.dram_tensor(
        "w", [128, (E // 128) * 3 * D], BF16, kind="ExternalInput"
    ).ap()
    bias_d = nc.dram_tensor("bias_cols", [D, 3], F32, kind="ExternalInput").ap()
    peer_d = nc.dram_tensor("peer", [1, 1], mybir.dt.uint32, kind="ExternalInput").ap()
    out_d = nc.dram_tensor("out_t", [D, HALF], F32, kind="ExternalOutput").ap()
    sums_d = nc.dram_tensor("sums", [1, HALF], F32, kind="ExternalOutput").ap()

    NE = E // 128  # 16 e-chunks
    NQB = HALF // QB  # 4 query blocks
    GROUPS = [[2 * i, 2 * i + 1] for i in range(N_CORES // 2)]

    SCH_A = float(SCALE * (1 << 7) / np.log(2.0))
    SCH_B = float(127 * (1 << 7) + 0.5 - 5.59)

    with tile.TileContext(nc) as tc:
        with (
            tc.tile_pool(name="xt", bufs=32) as xt_pool,
            tc.tile_pool(name="wsb", bufs=1) as w_pool,
            tc.tile_pool(name="persist", bufs=1) as persist,
            tc.tile_pool(name="vtt", bufs=2) as vtt_pool,
            tc.tile_pool(name="exp", bufs=20) as exp_pool,
            tc.tile_pool(name="comb", bufs=8) as comb_pool,
            tc.tile_pool(name="osb", bufs=2) as osb_pool,
            tc.tile_pool(name="dram", bufs=1, space="DRAM") as dram_pool,
            tc.tile_pool(name="ps_big", bufs=3, space="PSUM") as ps_big,
            tc.tile_pool(name="ps_acc", bufs=1, space="PSUM") as ps_acc,
            tc.tile_pool(name="ps_small", bufs=1, space="PSUM") as ps_small,
        ):
            # ---- constants ----
            bias_sb = persist.tile([D, 3], F32, tag="bias")
            nc.scalar.dma_start(bias_sb[:], bias_d[:])
            ones_col = persist.tile([128, 1], BF16, tag="ones")
            nc.gpsimd.memset(ones_col[:], 1.0)
            # preload the ACT Exp table now (~1.3us) instead of at the
            # first pass exp
            act_warm = persist.tile([1, 1], BF16, tag="act_warm")
            nc.scalar.activation(act_warm[:], bias_sb[0:1, 0:1], AF.Exp, scale=1.0)

            # ---- CC warmup (see module docstring) ----
            warm_in = dram_pool.tile([1, 2], BF16, tag="warm_in")
            warm_out = dram_pool.tile([2, 1, 2], BF16, tag="warm_out")
            nc.gpsimd.collective_compute(
                "AllGather",
                mybir.AluOpType.bypass,
                replica_groups=GROUPS,
                ins=[warm_in.opt()],
                outs=[warm_out.opt()],
            )

            # ---- w + x loads in consumption order ----
            w_sb = w_pool.tile([128, NE * 3 * D], BF16, tag="w")
            we = 3 * D
            wg = NE * 3 * D // 4  # w quarter piece: 4 e-chunks
            xt = {}
            nc.sync.dma_start(w_sb[:, 0:we], w_d[:, 0:we])
            nc.scalar.dma_start(w_sb[:, wg : 2 * wg], w_d[:, wg : 2 * wg])

            def load_x(sq, e):
                eng = nc.sync if e % 2 == 0 else nc.scalar
                t = xt_pool.tile([128, SQ], BF16, tag="xt", name=f"xt{sq}_{e}")
                eng.dma_start(
                    t[:], x_d[e * 128 : (e + 1) * 128, sq * SQ : (sq + 1) * SQ]
                )
                xt[(sq, e)] = t

            for e in range(NE):
                if e == 2:
                    nc.sync.dma_start(w_sb[:, we:wg], w_d[:, we:wg])
                if e == 4 or e == 5:
                    g = e - 2
                    nc.scalar.dma_start(
                        w_sb[:, g * wg : (g + 1) * wg], w_d[:, g * wg : (g + 1) * wg]
                    )
                load_x(0, e)
            for e in range(NE):
                load_x(1, e)

            # peer slot register (host supplies 1 on even cores, 0 on odd).
            # Allocated on gpsimd: the peer landings are gpsimd SWDGE DMAs
            # and register APs are engine-scoped.
            peer_reg = nc.gpsimd.alloc_register("peer_slot")
            nc.gpsimd.reg_load(peer_reg, peer_d[0:1, 0:1])
            peer_val = nc.gpsimd.snap(peer_reg, donate=True, min_val=0, max_val=1)

            qT = persist.tile([D, HALF], BF16, tag="qT")
            k_all = persist.tile([D, S], BF16, tag="k_all")  # [k own | k peer]
            v_sb = persist.tile([128, S // 128 * D], BF16, tag="v")  # own | peer
            sums_sb = persist.tile([1, HALF], F32, tag="sums_sb")
            o_stage = persist.tile([D, HALF], F32, tag="o_stage")

            # ---- collective staging (DRAM) ----
            cc_in = {}
            cc_out = {}
            for nm, shp in (
                ("k0", [D, SQ]),
                ("v0", [128, 8 * D]),
                ("k1", [D, SQ]),
                ("v1", [128, 8 * D]),
            ):
                cc_in[nm] = dram_pool.tile(
                    shp, BF16, tag=f"cc_in_{nm}", name=f"cc_in_{nm}"
                )
                cc_out[nm] = dram_pool.tile(
                    [2] + shp, BF16, tag=f"cc_out_{nm}", name=f"cc_out_{nm}"
                )

            def exchange(nm, src_ap):
                nc.gpsimd.dma_start(cc_in[nm][:], src_ap)
                nc.gpsimd.collective_compute(
                    "AllGather",
                    mybir.AluOpType.bypass,
                    replica_groups=GROUPS,
                    ins=[cc_in[nm].opt()],
                    outs=[cc_out[nm].opt()],
                )

            # ---- projection: per chunk K, V, Q; quarter at a time ----
            vt_tmp = [None, None]

            def transpose_unit(sq, j):
                """[128,128] vt_tmp[sq] -> v_sb chunk via DMA transpose.
                Bit-exact (verified standalone); rides the sync HWDGE ring,
                which is idle once the x stream drains — saves ~4.4us of PE
                identity-transposes plus the DVE PSUM->SBUF copies."""
                k = sq * 8 + j
                nc.sync.dma_start_transpose(
                    v_sb[:, k * D : (k + 1) * D],
                    vt_tmp[sq][:, j * 128 : (j + 1) * 128],
                )

            def dve_evac(dst_ap, ps, g):
                """PSUM -> SBUF with bias add, on the (projection-idle) DVE."""
                nc.vector.tensor_scalar_add(dst_ap, ps[:], bias_sb[:, g : g + 1])

            for sq in range(2):
                ps_k = ps_big.tile([128, SQ], F32, tag="ps_big", name=f"ps_k{sq}")
                ps_v = ps_big.tile([128, SQ], F32, tag="ps_big", name=f"ps_v{sq}")
                ps_q = ps_big.tile([128, SQ], F32, tag="ps_big", name=f"ps_q{sq}")
                for e in range(NE):
                    for g, ps in ((1, ps_k), (2, ps_v), (0, ps_q)):
                        w_ap = w_sb[:, e * 3 * D + g * D : e * 3 * D + (g + 1) * D]
                        for half in range(2):
                            nc.tensor.matmul(
                                ps[:, half * QB : (half + 1) * QB],
                                lhsT=w_ap,
                                rhs=xt[(sq, e)][:, half * QB : (half + 1) * QB],
                                start=(e == 0),
                                stop=(e == NE - 1),
                            )
                dve_evac(k_all[:, sq * SQ : (sq + 1) * SQ], ps_k, 1)
                vt_tmp[sq] = vtt_pool.tile([128, SQ], BF16, tag="vtt", name=f"vtt{sq}")
                dve_evac(vt_tmp[sq][:], ps_v, 2)
                dve_evac(qT[:, sq * SQ : (sq + 1) * SQ], ps_q, 0)
                for j in range(8):
                    transpose_unit(sq, j)
                if sq == 0:
                    exchange("k0", k_all[:, 0:SQ])

            exchange("v0", v_sb[:, 0 : 8 * D])
            exchange("k1", k_all[:, SQ:HALF])
            # v1's staging DMA is emitted after the stage-0 transposes below

            # peer landings, all on the (otherwise idle) gpsimd queue
            def land_peers():
                nc.gpsimd.dma_start(k_all[:, HALF : HALF + SQ], cc_out["k0"][peer_val])
                nc.gpsimd.dma_start(v_sb[:, 16 * D : 24 * D], cc_out["v0"][peer_val])
                nc.gpsimd.dma_start(k_all[:, HALF + SQ : S], cc_out["k1"][peer_val])
                nc.gpsimd.dma_start(v_sb[:, 24 * D : 32 * D], cc_out["v1"][peer_val])

            # ---- attention: software-pipelined blocks ----
            def scores_exp(qb, kp, on_dve):
                q_ap = qT[:, qb * QB : (qb + 1) * QB]
                ps_s = ps_big.tile([128, 2 * QB], F32, tag="ps_big")
                for half in range(2):
                    k = 2 * kp + half
                    nc.tensor.matmul(
                        ps_s[:, half * QB : (half + 1) * QB],
                        lhsT=k_all[:, k * 128 : (k + 1) * 128],
                        rhs=q_ap,
                        start=True,
                        stop=True,
                    )
                ex = exp_pool.tile([128, 2 * QB], BF16, tag="exp")
                if on_dve:
                    nc.vector.tensor_scalar(
                        ex[:].bitcast(mybir.dt.int16),
                        ps_s[:],
                        SCH_A,
                        SCH_B,
                        mybir.AluOpType.mult,
                        mybir.AluOpType.add,
                    )
                else:
                    nc.scalar.activation(ex[:], ps_s[:], AF.Exp, scale=SCALE)
                return ex

            def subtree(exs):
                level = list(exs)
                while len(level) > 1:
                    nxt = []
                    for i in range(0, len(level), 2):
                        if i + 1 < len(level):
                            comb = comb_pool.tile([128, 2 * QB], BF16, tag="comb")
                            nc.vector.tensor_add(comb[:], level[i][:], level[i + 1][:])
                            nxt.append(comb)
                        else:
                            nxt.append(level[i])
                    level = nxt
                return level[0]

            LEAF = 4
            # blocks: (qb, kp_list). Pass X = own keys, pass Y = peer keys;
            # the final block is split in two so the non-overlapped drain
            # (last PV group + ones + evac + store) covers 4 k-pairs, not 8.
            blocks = [(qb, list(range(0, 8))) for qb in range(NQB)]
            blocks += [(qb, list(range(8, 16))) for qb in range(NQB - 1)]
            blocks += [(3, [8, 9, 10, 11]), (3, [12, 13, 14, 15])]

            def emit_stage(cur, prev, extra_pe=None):
                """Interleave cur block's scores+exp with prev block's PV.
                extra_pe: optional per-step PE callables (stage-0 transposes).
                """
                if prev is not None:
                    prev["ps_o"] = ps_acc.tile([128, QB], F32, tag="ps_acc", name="ps_o")
                    prev["ps_sum"] = ps_small.tile(
                        [1, QB], F32, tag="ps_small", name="ps_sum"
                    )
                n_cur = len(cur["kps"]) if cur is not None else 0
                n_prev = len(prev["kps"]) if prev is not None else 0
                for i in range(max(n_cur, n_prev)):
                    if cur is not None and i < n_cur:
                        # DVE-Schraudolph on 1 of 8 exps: the DVE also owns
                        # the subtree adds and evacuations and saturates at
                        # 2/8 (measured ~100% busy through the pass)
                        cur["exs"].append(
                            scores_exp(cur["qb"], cur["kps"][i], on_dve=(i == 4))
                        )
                        if (i + 1) % LEAF == 0:
                            cur["roots"].append(
                                subtree(cur["exs"][i + 1 - LEAF : i + 1])
                            )
                    if extra_pe is not None and i < len(extra_pe):
                        extra_pe[i]()
                    if prev is not None and i < n_prev:
                        kp = prev["kps"][i]
                        for half in range(2):
                            k = 2 * kp + half
                            nc.tensor.matmul(
                                prev["ps_o"][:],
                                lhsT=v_sb[:, k * D : (k + 1) * D],
                                rhs=prev["exs"][i][:, half * QB : (half + 1) * QB],
                                start=(i == 0 and half == 0),
                                stop=(i == n_prev - 1 and half == 1),
                            )
                if prev is None:
                    return
                for ri, root in enumerate(prev["roots"]):
                    for half in range(2):
                        nc.tensor.matmul(
                            prev["ps_sum"][:],
                            lhsT=ones_col[:],
                            rhs=root[:, half * QB : (half + 1) * QB],
                            start=(ri == 0 and half == 0),
                            stop=(ri == len(prev["roots"]) - 1 and half == 1),
                        )
                qb = prev["qb"]
                o_sl = o_stage[:, qb * QB : (qb + 1) * QB]
                s_sl = sums_sb[:, qb * QB : (qb + 1) * QB]
                if prev["kps"][0] == 0:  # pass X: stage into SBUF
                    nc.vector.tensor_copy(o_sl, prev["ps_o"][:])
                    nc.vector.tensor_copy(s_sl, prev["ps_sum"][:])
                elif not prev["last"]:  # pass Y, partial: accumulate in place
                    nc.vector.tensor_add(o_sl, o_sl, prev["ps_o"][:])
                    nc.vector.tensor_add(s_sl, s_sl, prev["ps_sum"][:])
                else:  # final contribution for this qb: combine + store
                    o_out = osb_pool.tile([128, QB], F32, tag="osb")
                    nc.vector.tensor_add(o_out[:], o_sl, prev["ps_o"][:])
                    nc.vector.tensor_add(s_sl, s_sl, prev["ps_sum"][:])
                    nc.sync.dma_start(out_d[:, qb * QB : (qb + 1) * QB], o_out[:])
                    nc.sync.dma_start(sums_d[:, qb * QB : (qb + 1) * QB], s_sl)

            prev = None
            for bi, (qb, kps) in enumerate(blocks):
                cur = {
                    "qb": qb,
                    "kps": kps,
                    "exs": [],
                    "roots": [],
                    "last": (kps[-1] == 15),
                }
                emit_stage(cur, prev)
                if bi == 0:
                    exchange("v1", v_sb[:, 8 * D : 16 * D])
                    land_peers()
                prev = cur
            emit_stage(None, prev)

    nc.compile()
    return nc


def _prep_inputs(x, W, b):
    """Host-side sharding prep: cast bf16, transpose to xT, slice halves."""
    b_f = np.asarray(b, dtype=np.float32)
    bias_cols = np.ascontiguousarray(b_f.reshape(3, D).T)  # [128, 3]
    w_bf = np.ascontiguousarray(
        np.asarray(W)
        .astype(ml_dtypes.bfloat16)
        .reshape(E // 128, 128, 3 * D)
        .transpose(1, 0, 2)
        .reshape(128, (E // 128) * 3 * D)
    )
    in_maps = []
    for bb in range(B):
        xt_full = np.ascontiguousarray(
            np.asarray(x[bb]).astype(ml_dtypes.bfloat16).T
        )  # [E, S]
        for h in range(2):
            xc = np.ascontiguousarray(xt_full[:, h * HALF : (h + 1) * HALF])
            peer = np.array([[1 - h]], dtype=np.uint32)
            in_maps.append(
                {"xt": xc, "w": w_bf, "bias_cols": bias_cols, "peer": peer}
            )
    return in_maps


def _run(in_maps, trace=False, trace_kwargs=None):
    if "nc" not in _CACHE:
        _CACHE["nc"] = _build()
    return run_bass_kernel_spmd(
        _CACHE["nc"],
        in_maps,
        list(range(N_CORES)),
        trace=trace,
        **(trace_kwargs or {}),
    )


def kernel(x, W, b):
    in_maps = _prep_inputs(x, W, b)
    res = None
    for attempt in range(3):
        try:
            res = _run(in_maps)
            break
        except Exception:
            if attempt == 2:
                raise
    out = np.empty((B, S, D), dtype=np.float32)
    for c in range(N_CORES):
        bb, h = c // 2, c % 2
        o_t = res.results[c]["out_t"]  # [D, HALF]
        sums = res.results[c]["sums"]  # [1, HALF]
        out[bb, h * HALF : (h + 1) * HALF, :] = (o_t / sums).T
    return out


# revision 12
# speedup vs baseline: 1.1163x; 1.1163x over previous
"""Single-head attention (B=4, S=4096, E=2048, d=128) on 8 trn2 cores.

Sharding: core c handles (batch b = c//2, seq half h = c%2). Each core
projects q/k/v for its own 2048-row half; the pair (2b, 2b+1) exchanges
K and V halves via four pairwise AllGathers. Measured CC behavior: the
first mesh cannot begin before ~52us regardless of trigger time (NRT
arming), then meshes run serially at ~7-8us per 256KB — so a dummy
warmup AllGather is fired at ~8us to absorb the arming latency, and the
attention pass is split own/peer (8/8 k-pairs) so peer data is first
consumed ~95us, far behind the worst-case exchange completion (~85us).

Engine/queue plan (all measured):
  sync HWDGE ring: w pieces 0/1, x even e-chunks (both quarters),
    output stores. scalar HWDGE ring: bias, w pieces 2-4, x odd
    e-chunks. Two rings share ~360GB/s; a third (gpsimd SWDGE) path
    starves the scalar ring, so x stays on two rings.
  ACT queue: exp only (plus a tiny warmup activation to preload the
    Exp table before the pass). Projection evacuations run on the DVE
    (tensor_scalar_add with the [128,1] bias column) — ACT evacuations
    behind ring-credit-gated DMA issues measurably slip by >10us.
  gpsimd queue: collective staging DMAs in AND out + triggers. (cc_out
    landings on the sync ring got statically scheduled after pass
    stores, stalling pass-Y PV by ~4us.)

Projection: per streamed x chunk, K, V, Q matmuls (6 x N=512) per
quarter — PE-bound at ~1.28us/chunk vs ~0.72us arrival. V-half
transposes ([d,k]->[k,d] PE identity transposes): quarter-0's ride the
quarter-1 matmul stream, quarter-1's ride the first pass stage.
PSUM: ps_big 3x[128,1024] holds K,V,Q of one quarter, rotating into
the next quarter then score tiles; ps_acc/ps_small hold transposes
during projection, ps_o/ps_sum during the passes.

Attention: blocks of 8 k-pairs x 512 queries, software-pipelined:
block n's score matmuls interleave with block n-1's PV matmuls in the
PE FIFO, so the exp engines (ACT 6 + DVE-Schraudolph 2 per block, ~3%
max rel err, numerically validated to 1/2 of tiles) stay under the
~7.9us PE stage time and score PSUM tiles recycle with slack.
Denominators: DVE pair-add subtrees (leaf=4) at leaf boundaries +
exact ones-column matmuls after the PVs. Output/sums evacuate on the
DVE; host divides and transposes.
"""

import numpy as np
import ml_dtypes

import concourse.tile as tile
from concourse import bacc, mybir
from concourse.bass_utils import run_bass_kernel_spmd
from concourse.masks import make_identity

N_CORES = 8
B, S, E, D = 4, 4096, 2048, 128
HALF = S // 2  # queries / own keys per core
QB = 512  # query block (PSUM bank width in fp32)
SQ = 1024  # projection quarter width
SCALE = 1.0 / float(np.sqrt(D))

BF16 = mybir.dt.bfloat16
F32 = mybir.dt.float32
AF = mybir.ActivationFunctionType

_CACHE = {}


def _build():
    nc = bacc.Bacc(
        trn_type="TRN2", target_bir_lowering=False, debug=False, num_devices=N_CORES
    )

    x_d = nc.dram_tensor("xt", [E, HALF], BF16, kind="ExternalInput").ap()
    w_d = nc.dram_tensor(
        "w", [128, (E // 128) * 3 * D], BF16, kind="ExternalInput"
    ).ap()
    bias_d = nc.dram_tensor("bias_cols", [D, 3], F32, kind="ExternalInput").ap()
    peer_d = nc.dram_tensor("peer", [1, 1], mybir.dt.uint32, kind="ExternalInput").ap()
    out_d = nc.dram_tensor("out_t", [D, HALF], F32, kind="ExternalOutput").ap()
    sums_d = nc.dram_tensor("sums", [1, HALF], F32, kind="ExternalOutput").ap()

    NE = E // 128  # 16 e-chunks
    NQB = HALF // QB  # 4 query blocks
    GROUPS = [[2 * i, 2 * i + 1] for i in range(N_CORES // 2)]

    SCH_A = float(SCALE * (1 << 7) / np.log(2.0))
    SCH_B = float(127 * (1 << 7) + 0.5 - 5.59)

    with tile.TileContext(nc) as tc:
        with (
            tc.tile_pool(name="xt", bufs=32) as xt_pool,
            tc.tile_pool(name="wsb", bufs=1) as w_pool,
            tc.tile_pool(name="persist", bufs=1) as persist,
            tc.tile_pool(name="vtt", bufs=2) as vtt_pool,
            tc.tile_pool(name="exp", bufs=20) as exp_pool,
            tc.tile_pool(name="comb", bufs=8) as comb_pool,
            tc.tile_pool(name="osb", bufs=2) as osb_pool,
            tc.tile_pool(name="dram", bufs=1, space="DRAM") as dram_pool,
            tc.tile_pool(name="ps_big", bufs=3, space="PSUM") as ps_big,
            tc.tile_pool(name="ps_acc", bufs=1, space="PSUM") as ps_acc,
            tc.tile_pool(name="ps_small", bufs=1, space="PSUM") as ps_small,
        ):
            # ---- constants ----
            bias_sb = persist.tile([D, 3], F32, tag="bias")
            nc.scalar.dma_start(bias_sb[:], bias_d[:])
            ones_col = persist.tile([128, 1], BF16, tag="ones")
            nc.gpsimd.memset(ones_col[:], 1.0)
            ident = persist.tile([128, 128], BF16, tag="ident")
            make_identity(nc, ident[:])
            # preload the ACT Exp table now (~1.3us) instead of at the
            # first pass exp
            act_warm = persist.tile([1, 1], BF16, tag="act_warm")
            nc.scalar.activation(act_warm[:], bias_sb[0:1, 0:1], AF.Exp, scale=1.0)

            # ---- CC warmup (see module docstring) ----
            warm_in = dram_pool.tile([1, 2], BF16, tag="warm_in")
            warm_out = dram_pool.tile([2, 1, 2], BF16, tag="warm_out")
            nc.gpsimd.collective_compute(
                "AllGather",
                mybir.AluOpType.bypass,
                replica_groups=GROUPS,
                ins=[warm_in.opt()],
                outs=[warm_out.opt()],
            )

            # ---- w + x loads in consumption order ----
            w_sb = w_pool.tile([128, NE * 3 * D], BF16, tag="w")
            we = 3 * D
            wg = NE * 3 * D // 4  # w quarter piece: 4 e-chunks
            xt = {}
            nc.sync.dma_start(w_sb[:, 0:we], w_d[:, 0:we])
            nc.scalar.dma_start(w_sb[:, wg : 2 * wg], w_d[:, wg : 2 * wg])

            def load_x(sq, e):
                eng = nc.sync if e % 2 == 0 else nc.scalar
                t = xt_pool.tile([128, SQ], BF16, tag="xt", name=f"xt{sq}_{e}")
                eng.dma_start(
                    t[:], x_d[e * 128 : (e + 1) * 128, sq * SQ : (sq + 1) * SQ]
                )
                xt[(sq, e)] = t

            for e in range(NE):
                if e == 2:
                    nc.sync.dma_start(w_sb[:, we:wg], w_d[:, we:wg])
                if e == 4 or e == 5:
                    g = e - 2
                    nc.scalar.dma_start(
                        w_sb[:, g * wg : (g + 1) * wg], w_d[:, g * wg : (g + 1) * wg]
                    )
                load_x(0, e)
            for e in range(NE):
                load_x(1, e)

            # peer slot register (host supplies 1 on even cores, 0 on odd).
            # Allocated on gpsimd: the peer landings are gpsimd SWDGE DMAs
            # and register APs are engine-scoped.
            peer_reg = nc.gpsimd.alloc_register("peer_slot")
            nc.gpsimd.reg_load(peer_reg, peer_d[0:1, 0:1])
            peer_val = nc.gpsimd.snap(peer_reg, donate=True, min_val=0, max_val=1)

            qT = persist.tile([D, HALF], BF16, tag="qT")
            k_all = persist.tile([D, S], BF16, tag="k_all")  # [k own | k peer]
            v_sb = persist.tile([128, S // 128 * D], BF16, tag="v")  # own | peer
            sums_sb = persist.tile([1, HALF], F32, tag="sums_sb")
            o_stage = persist.tile([D, HALF], F32, tag="o_stage")

            # ---- collective staging (DRAM) ----
            cc_in = {}
            cc_out = {}
            for nm, shp in (
                ("k0", [D, SQ]),
                ("v0", [128, 8 * D]),
                ("k1", [D, SQ]),
                ("v1", [128, 8 * D]),
            ):
                cc_in[nm] = dram_pool.tile(
                    shp, BF16, tag=f"cc_in_{nm}", name=f"cc_in_{nm}"
                )
                cc_out[nm] = dram_pool.tile(
                    [2] + shp, BF16, tag=f"cc_out_{nm}", name=f"cc_out_{nm}"
                )

            def exchange(nm, src_ap):
                nc.gpsimd.dma_start(cc_in[nm][:], src_ap)
                nc.gpsimd.collective_compute(
                    "AllGather",
                    mybir.AluOpType.bypass,
                    replica_groups=GROUPS,
                    ins=[cc_in[nm].opt()],
                    outs=[cc_out[nm].opt()],
                )

            # ---- projection: per chunk K, V, Q; quarter at a time ----
            vt_tmp = [None, None]

            def transpose_unit(sq, j):
                """One [128,128] PE transpose of vt_tmp[sq] -> v_sb chunk.
                (DMA-transpose was tried: bit-exact but ~7us per 32KB tile
                through the xbar path — useless here.)"""
                pool, ptag = (ps_acc, "ps_acc") if j % 2 == 0 else (ps_small, "ps_small")
                ps_t = pool.tile([128, 128], BF16, tag=ptag)
                nc.tensor.transpose(
                    ps_t[:], vt_tmp[sq][:, j * 128 : (j + 1) * 128], ident[:]
                )
                k = sq * 8 + j
                nc.vector.tensor_copy(v_sb[:, k * D : (k + 1) * D], ps_t[:])

            def dve_evac(dst_ap, ps, g):
                """PSUM -> SBUF with bias add, on the (projection-idle) DVE."""
                nc.vector.tensor_scalar_add(dst_ap, ps[:], bias_sb[:, g : g + 1])

            for sq in range(2):
                ps_k = ps_big.tile([128, SQ], F32, tag="ps_big", name=f"ps_k{sq}")
                ps_v = ps_big.tile([128, SQ], F32, tag="ps_big", name=f"ps_v{sq}")
                ps_q = ps_big.tile([128, SQ], F32, tag="ps_big", name=f"ps_q{sq}")
                for e in range(NE):
                    for g, ps in ((1, ps_k), (2, ps_v), (0, ps_q)):
                        w_ap = w_sb[:, e * 3 * D + g * D : e * 3 * D + (g + 1) * D]
                        for half in range(2):
                            nc.tensor.matmul(
                                ps[:, half * QB : (half + 1) * QB],
                                lhsT=w_ap,
                                rhs=xt[(sq, e)][:, half * QB : (half + 1) * QB],
                                start=(e == 0),
                                stop=(e == NE - 1),
                            )
                    if sq == 1 and e < 8:
                        transpose_unit(0, e)  # quarter-0 V transposes ride here
                dve_evac(k_all[:, sq * SQ : (sq + 1) * SQ], ps_k, 1)
                vt_tmp[sq] = vtt_pool.tile([128, SQ], BF16, tag="vtt", name=f"vtt{sq}")
                dve_evac(vt_tmp[sq][:], ps_v, 2)
                dve_evac(qT[:, sq * SQ : (sq + 1) * SQ], ps_q, 0)
                if sq == 0:
                    exchange("k0", k_all[:, 0:SQ])

            exchange("v0", v_sb[:, 0 : 8 * D])
            exchange("k1", k_all[:, SQ:HALF])
            # v1's staging DMA is emitted after the stage-0 transposes below

            # peer landings, all on the (otherwise idle) gpsimd queue
            def land_peers():
                nc.gpsimd.dma_start(k_all[:, HALF : HALF + SQ], cc_out["k0"][peer_val])
                nc.gpsimd.dma_start(v_sb[:, 16 * D : 24 * D], cc_out["v0"][peer_val])
                nc.gpsimd.dma_start(k_all[:, HALF + SQ : S], cc_out["k1"][peer_val])
                nc.gpsimd.dma_start(v_sb[:, 24 * D : 32 * D], cc_out["v1"][peer_val])

            # ---- attention: software-pipelined blocks ----
            def scores_exp(qb, kp, on_dve):
                q_ap = qT[:, qb * QB : (qb + 1) * QB]
                ps_s = ps_big.tile([128, 2 * QB], F32, tag="ps_big")
                for half in range(2):
                    k = 2 * kp + half
                    nc.tensor.matmul(
                        ps_s[:, half * QB : (half + 1) * QB],
                        lhsT=k_all[:, k * 128 : (k + 1) * 128],
                        rhs=q_ap,
                        start=True,
                        stop=True,
                    )
                ex = exp_pool.tile([128, 2 * QB], BF16, tag="exp")
                if on_dve:
                    nc.vector.tensor_scalar(
                        ex[:].bitcast(mybir.dt.int16),
                        ps_s[:],
                        SCH_A,
                        SCH_B,
                        mybir.AluOpType.mult,
                        mybir.AluOpType.add,
                    )
                else:
                    nc.scalar.activation(ex[:], ps_s[:], AF.Exp, scale=SCALE)
                return ex

            def subtree(exs):
                level = list(exs)
                while len(level) > 1:
                    nxt = []
                    for i in range(0, len(level), 2):
                        if i + 1 < len(level):
                            comb = comb_pool.tile([128, 2 * QB], BF16, tag="comb")
                            nc.vector.tensor_add(comb[:], level[i][:], level[i + 1][:])
                            nxt.append(comb)
                        else:
                            nxt.append(level[i])
                    level = nxt
                return level[0]

            LEAF = 4
            # blocks: (qb, kp_list). Pass X = own keys, pass Y = peer keys;
            # the final block is split in two so the non-overlapped drain
            # (last PV group + ones + evac + store) covers 4 k-pairs, not 8.
            blocks = [(qb, list(range(0, 8))) for qb in range(NQB)]
            blocks += [(qb, list(range(8, 16))) for qb in range(NQB - 1)]
            blocks += [(3, [8, 9, 10, 11]), (3, [12, 13, 14, 15])]

            def emit_stage(cur, prev, extra_pe=None):
                """Interleave cur block's scores+exp with prev block's PV.
                extra_pe: optional per-step PE callables (stage-0 transposes).
                """
                if prev is not None:
                    prev["ps_o"] = ps_acc.tile([128, QB], F32, tag="ps_acc", name="ps_o")
                    prev["ps_sum"] = ps_small.tile(
                        [1, QB], F32, tag="ps_small", name="ps_sum"
                    )
                n_cur = len(cur["kps"]) if cur is not None else 0
                n_prev = len(prev["kps"]) if prev is not None else 0
                for i in range(max(n_cur, n_prev)):
                    if cur is not None and i < n_cur:
                        # DVE-Schraudolph on 1 of 8 exps: the DVE also owns
                        # the subtree adds and evacuations and saturates at
                        # 2/8 (measured ~100% busy through the pass)
                        cur["exs"].append(
                            scores_exp(cur["qb"], cur["kps"][i], on_dve=(i == 4))
                        )
                        if (i + 1) % LEAF == 0:
                            cur["roots"].append(
                                subtree(cur["exs"][i + 1 - LEAF : i + 1])
                            )
                    if extra_pe is not None and i < len(extra_pe):
                        extra_pe[i]()
                    if prev is not None and i < n_prev:
                        kp = prev["kps"][i]
                        for half in range(2):
                            k = 2 * kp + half
                            nc.tensor.matmul(
                                prev["ps_o"][:],
                                lhsT=v_sb[:, k * D : (k + 1) * D],
                                rhs=prev["exs"][i][:, half * QB : (half + 1) * QB],
                                start=(i == 0 and half == 0),
                                stop=(i == n_prev - 1 and half == 1),
                            )
                if prev is None:
                    return
                for ri, root in enumerate(prev["roots"]):
                    for half in range(2):
                        nc.tensor.matmul(
                            prev["ps_sum"][:],
                            lhsT=ones_col[:],
                            rhs=root[:, half * QB : (half + 1) * QB],
                            start=(ri == 0 and half == 0),
                            stop=(ri == len(prev["roots"]) - 1 and half == 1),
                        )
                qb = prev["qb"]
                o_sl = o_stage[:, qb * QB : (qb + 1) * QB]
                s_sl = sums_sb[:, qb * QB : (qb + 1) * QB]
                if prev["kps"][0] == 0:  # pass X: stage into SBUF
                    nc.vector.tensor_copy(o_sl, prev["ps_o"][:])
                    nc.vector.tensor_copy(s_sl, prev["ps_sum"][:])
                elif not prev["last"]:  # pass Y, partial: accumulate in place
                    nc.vector.tensor_add(o_sl, o_sl, prev["ps_o"][:])
                    nc.vector.tensor_add(s_sl, s_sl, prev["ps_sum"][:])
                else:  # final contribution for this qb: combine + store
                    o_out = osb_pool.tile([128, QB], F32, tag="osb")
                    nc.vector.tensor_add(o_out[:], o_sl, prev["ps_o"][:])
                    nc.vector.tensor_add(s_sl, s_sl, prev["ps_sum"][:])
                    nc.sync.dma_start(out_d[:, qb * QB : (qb + 1) * QB], o_out[:])
                    nc.sync.dma_start(sums_d[:, qb * QB : (qb + 1) * QB], s_sl)

            prev = None
            for bi, (qb, kps) in enumerate(blocks):
                cur = {
                    "qb": qb,
                    "kps": kps,
                    "exs": [],
                    "roots": [],
                    "last": (kps[-1] == 15),
                }
                extra = None
                if bi == 0:
                    # quarter-1 V transposes ride the first (PV-less) stage
                    extra = [
                        (lambda j=j: transpose_unit(1, j)) for j in range(8)
                    ]
                emit_stage(cur, prev, extra_pe=extra)
                if bi == 0:
                    exchange("v1", v_sb[:, 8 * D : 16 * D])
                    land_peers()
                prev = cur
            emit_stage(None, prev)

    nc.compile()
    return nc


def _prep_inputs(x, W, b):
    """Host-side sharding prep: cast bf16, transpose to xT, slice halves."""
    b_f = np.asarray(b, dtype=np.float32)
    bias_cols = np.ascontiguousarray(b_f.reshape(3, D).T)  # [128, 3]
    w_bf = np.ascontiguousarray(
        np.asarray(W)
        .astype(ml_dtypes.bfloat16)
        .reshape(E // 128, 128, 3 * D)
        .transpose(1, 0, 2)
        .reshape(128, (E // 128) * 3 * D)
    )
    in_maps = []
    for bb in range(B):
        xt_full = np.ascontiguousarray(
            np.asarray(x[bb]).astype(ml_dtypes.bfloat16).T
        )  # [E, S]
        for h in range(2):
            xc = np.ascontiguousarray(xt_full[:, h * HALF : (h + 1) * HALF])
            peer = np.array([[1 - h]], dtype=np.uint32)
            in_maps.append(
                {"xt": xc, "w": w_bf, "bias_cols": bias_cols, "peer": peer}
            )
    return in_maps


def _run(in_maps, trace=False, trace_kwargs=None):
    if "nc" not in _CACHE:
        _CACHE["nc"] = _build()
    return run_bass_kernel_spmd(
        _CACHE["nc"],
        in_maps,
        list(range(N_CORES)),
        trace=trace,
        **(trace_kwargs or {}),
    )


def kernel(x, W, b):
    in_maps = _prep_inputs(x, W, b)
    res = None
    for attempt in range(3):
        try:
            res = _run(in_maps)
            break
        except Exception:
            if attempt == 2:
                raise
    out = np.empty((B, S, D), dtype=np.float32)
    for c in range(N_CORES):
        bb, h = c // 2, c % 2
        o_t = res.results[c]["out_t"]  # [D, HALF]
        sums = res.results[c]["sums"]  # [1, HALF]
        out[bb, h * HALF : (h + 1) * HALF, :] = (o_t / sums).T
    return out


# revision 13
# speedup vs baseline: 1.1169x; 1.0005x over previous
"""Single-head attention (B=4, S=4096, E=2048, d=128) on 8 trn2 cores.

Sharding: core c handles (batch b = c//2, seq half h = c%2). Each core
projects q/k/v for its own 2048-row half; the pair (2b, 2b+1) exchanges
K and V halves via four pairwise AllGathers. Measured CC behavior: the
first mesh cannot begin before ~52us regardless of trigger time (NRT
arming), then meshes run serially at ~7-8us per 256KB — so a dummy
warmup AllGather is fired at ~8us to absorb the arming latency, and the
attention pass is split own/peer (8/8 k-pairs) so peer data is first
consumed ~95us, far behind the worst-case exchange completion (~85us).

Engine/queue plan (all measured):
  sync HWDGE ring: w pieces 0/1, x even e-chunks (both quarters),
    output stores. scalar HWDGE ring: bias, w pieces 2-4, x odd
    e-chunks. Two rings share ~360GB/s; a third (gpsimd SWDGE) path
    starves the scalar ring, so x stays on two rings.
  ACT queue: exp only (plus a tiny warmup activation to preload the
    Exp table before the pass). Projection evacuations run on the DVE
    (tensor_scalar_add with the [128,1] bias column) — ACT evacuations
    behind ring-credit-gated DMA issues measurably slip by >10us.
  gpsimd queue: collective staging DMAs in AND out + triggers. (cc_out
    landings on the sync ring got statically scheduled after pass
    stores, stalling pass-Y PV by ~4us.)

Projection: per streamed x chunk, K, V, Q matmuls (6 x N=512) per
quarter — PE-bound at ~1.28us/chunk vs ~0.72us arrival. V-half
transposes ([d,k]->[k,d] PE identity transposes): quarter-0's ride the
quarter-1 matmul stream, quarter-1's ride the first pass stage.
PSUM: ps_big 3x[128,1024] holds K,V,Q of one quarter, rotating into
the next quarter then score tiles; ps_acc/ps_small hold transposes
during projection, ps_o/ps_sum during the passes.

Attention: blocks of 8 k-pairs x 512 queries, software-pipelined:
block n's score matmuls interleave with block n-1's PV matmuls in the
PE FIFO, so the exp engines (ACT 6 + DVE-Schraudolph 2 per block, ~3%
max rel err, numerically validated to 1/2 of tiles) stay under the
~7.9us PE stage time and score PSUM tiles recycle with slack.
Denominators: DVE pair-add subtrees (leaf=4) at leaf boundaries +
exact ones-column matmuls after the PVs. Output/sums evacuate on the
DVE; host divides and transposes.
"""

import numpy as np
import ml_dtypes

import concourse.tile as tile
from concourse import bacc, mybir
from concourse.bass_utils import run_bass_kernel_spmd
from concourse.masks import make_identity

N_CORES = 8
B, S, E, D = 4, 4096, 2048, 128
HALF = S // 2  # queries / own keys per core
QB = 512  # query block (PSUM bank width in fp32)
SQ = 1024  # projection quarter width
SCALE = 1.0 / float(np.sqrt(D))

BF16 = mybir.dt.bfloat16
F32 = mybir.dt.float32
AF = mybir.ActivationFunctionType

_CACHE = {}


def _build():
    nc = bacc.Bacc(
        trn_type="TRN2", target_bir_lowering=False, debug=False, num_devices=N_CORES
    )

    x_d = nc.dram_tensor("xt", [E, HALF], BF16, kind="ExternalInput").ap()
    w_d = nc.dram_tensor(
        "w", [128, (E // 128) * 3 * D], BF16, kind="ExternalInput"
    ).ap()
    bias_d = nc.dram_tensor("bias_cols", [D, 3], F32, kind="ExternalInput").ap()
    peer_d = nc.dram_tensor("peer", [1, 1], mybir.dt.uint32, kind="ExternalInput").ap()
    out_d = nc.dram_tensor("out_t", [D, HALF], F32, kind="ExternalOutput").ap()
    sums_d = nc.dram_tensor("sums", [1, HALF], F32, kind="ExternalOutput").ap()

    NE = E // 128  # 16 e-chunks
    NQB = HALF // QB  # 4 query blocks
    GROUPS = [[2 * i, 2 * i + 1] for i in range(N_CORES // 2)]

    SCH_A = float(SCALE * (1 << 7) / np.log(2.0))
    SCH_B = float(127 * (1 << 7) + 0.5 - 5.59)

    with tile.TileContext(nc) as tc:
        with (
            tc.tile_pool(name="xt", bufs=32) as xt_pool,
            tc.tile_pool(name="wsb", bufs=1) as w_pool,
            tc.tile_pool(name="persist", bufs=1) as persist,
            tc.tile_pool(name="vtt", bufs=2) as vtt_pool,
            tc.tile_pool(name="exp", bufs=20) as exp_pool,
            tc.tile_pool(name="comb", bufs=8) as comb_pool,
            tc.tile_pool(name="osb", bufs=2) as osb_pool,
            tc.tile_pool(name="dram", bufs=1, space="DRAM") as dram_pool,
            tc.tile_pool(name="ps_big", bufs=3, space="PSUM") as ps_big,
            tc.tile_pool(name="ps_acc", bufs=1, space="PSUM") as ps_acc,
            tc.tile_pool(name="ps_small", bufs=1, space="PSUM") as ps_small,
        ):
            # ---- constants ----
            bias_sb = persist.tile([D, 3], F32, tag="bias")
            nc.scalar.dma_start(bias_sb[:], bias_d[:])
            ones_col = persist.tile([128, 1], BF16, tag="ones")
            nc.gpsimd.memset(ones_col[:], 1.0)
            ident = persist.tile([128, 128], BF16, tag="ident")
            make_identity(nc, ident[:])
            # preload the ACT Exp table now (~1.3us) instead of at the
            # first pass exp
            act_warm = persist.tile([1, 1], BF16, tag="act_warm")
            nc.scalar.activation(act_warm[:], bias_sb[0:1, 0:1], AF.Exp, scale=1.0)

            # ---- CC warmup (see module docstring) ----
            warm_in = dram_pool.tile([1, 2], BF16, tag="warm_in")
            warm_out = dram_pool.tile([2, 1, 2], BF16, tag="warm_out")
            nc.gpsimd.collective_compute(
                "AllGather",
                mybir.AluOpType.bypass,
                replica_groups=GROUPS,
                ins=[warm_in.opt()],
                outs=[warm_out.opt()],
            )

            # ---- w + x loads in consumption order ----
            w_sb = w_pool.tile([128, NE * 3 * D], BF16, tag="w")
            we = 3 * D
            wg = NE * 3 * D // 4  # w quarter piece: 4 e-chunks
            xt = {}
            nc.sync.dma_start(w_sb[:, 0:we], w_d[:, 0:we])
            nc.scalar.dma_start(w_sb[:, wg : 2 * wg], w_d[:, wg : 2 * wg])

            def load_x(sq, e):
                eng = nc.sync if e % 2 == 0 else nc.scalar
                t = xt_pool.tile([128, SQ], BF16, tag="xt", name=f"xt{sq}_{e}")
                eng.dma_start(
                    t[:], x_d[e * 128 : (e + 1) * 128, sq * SQ : (sq + 1) * SQ]
                )
                xt[(sq, e)] = t

            for e in range(NE):
                if e == 2:
                    nc.sync.dma_start(w_sb[:, we:wg], w_d[:, we:wg])
                if e == 4 or e == 5:
                    g = e - 2
                    nc.scalar.dma_start(
                        w_sb[:, g * wg : (g + 1) * wg], w_d[:, g * wg : (g + 1) * wg]
                    )
                load_x(0, e)
            for e in range(NE):
                load_x(1, e)

            # peer slot register (host supplies 1 on even cores, 0 on odd).
            # Allocated on gpsimd: the peer landings are gpsimd SWDGE DMAs
            # and register APs are engine-scoped.
            peer_reg = nc.gpsimd.alloc_register("peer_slot")
            nc.gpsimd.reg_load(peer_reg, peer_d[0:1, 0:1])
            peer_val = nc.gpsimd.snap(peer_reg, donate=True, min_val=0, max_val=1)

            qT = persist.tile([D, HALF], BF16, tag="qT")
            k_all = persist.tile([D, S], BF16, tag="k_all")  # [k own | k peer]
            v_sb = persist.tile([128, S // 128 * D], BF16, tag="v")  # own | peer
            sums_sb = persist.tile([1, HALF], F32, tag="sums_sb")
            o_stage = persist.tile([D, HALF], F32, tag="o_stage")

            # ---- collective staging (DRAM) ----
            cc_in = {}
            cc_out = {}
            for nm, shp in (
                ("k0", [D, SQ]),
                ("v0", [128, 8 * D]),
                ("k1", [D, SQ]),
                ("v1", [128, 8 * D]),
            ):
                cc_in[nm] = dram_pool.tile(
                    shp, BF16, tag=f"cc_in_{nm}", name=f"cc_in_{nm}"
                )
                cc_out[nm] = dram_pool.tile(
                    [2] + shp, BF16, tag=f"cc_out_{nm}", name=f"cc_out_{nm}"
                )

            def exchange(nm, src_ap):
                nc.gpsimd.dma_start(cc_in[nm][:], src_ap)
                nc.gpsimd.collective_compute(
                    "AllGather",
                    mybir.AluOpType.bypass,
                    replica_groups=GROUPS,
                    ins=[cc_in[nm].opt()],
                    outs=[cc_out[nm].opt()],
                )

            # ---- projection: per chunk K, V, Q; quarter at a time ----
            vt_tmp = [None, None]

            def transpose_unit(sq, j):
                """One [128,128] PE transpose of vt_tmp[sq] -> v_sb chunk.
                (DMA-transpose was tried: bit-exact but ~7us per 32KB tile
                through the xbar path — useless here.)"""
                pool, ptag = (ps_acc, "ps_acc") if j % 2 == 0 else (ps_small, "ps_small")
                ps_t = pool.tile([128, 128], BF16, tag=ptag)
                nc.tensor.transpose(
                    ps_t[:], vt_tmp[sq][:, j * 128 : (j + 1) * 128], ident[:]
                )
                k = sq * 8 + j
                nc.vector.tensor_copy(v_sb[:, k * D : (k + 1) * D], ps_t[:])

            def dve_evac(dst_ap, ps, g):
                """PSUM -> SBUF with bias add, on the (projection-idle) DVE."""
                nc.vector.tensor_scalar_add(dst_ap, ps[:], bias_sb[:, g : g + 1])

            for sq in range(2):
                ps_k = ps_big.tile([128, SQ], F32, tag="ps_big", name=f"ps_k{sq}")
                ps_v = ps_big.tile([128, SQ], F32, tag="ps_big", name=f"ps_v{sq}")
                ps_q = ps_big.tile([128, SQ], F32, tag="ps_big", name=f"ps_q{sq}")
                for e in range(NE):
                    for g, ps in ((1, ps_k), (2, ps_v), (0, ps_q)):
                        w_ap = w_sb[:, e * 3 * D + g * D : e * 3 * D + (g + 1) * D]
                        for half in range(2):
                            nc.tensor.matmul(
                                ps[:, half * QB : (half + 1) * QB],
                                lhsT=w_ap,
                                rhs=xt[(sq, e)][:, half * QB : (half + 1) * QB],
                                start=(e == 0),
                                stop=(e == NE - 1),
                            )
                    if sq == 1 and e < 8:
                        transpose_unit(0, e)  # quarter-0 V transposes ride here
                dve_evac(k_all[:, sq * SQ : (sq + 1) * SQ], ps_k, 1)
                vt_tmp[sq] = vtt_pool.tile([128, SQ], BF16, tag="vtt", name=f"vtt{sq}")
                dve_evac(vt_tmp[sq][:], ps_v, 2)
                dve_evac(qT[:, sq * SQ : (sq + 1) * SQ], ps_q, 0)
                if sq == 0:
                    exchange("k0", k_all[:, 0:SQ])

            exchange("v0", v_sb[:, 0 : 8 * D])
            exchange("k1", k_all[:, SQ:HALF])
            # v1's staging DMA is emitted after the stage-0 transposes below

            # peer landings, all on the (otherwise idle) gpsimd queue
            def land_peers():
                nc.gpsimd.dma_start(k_all[:, HALF : HALF + SQ], cc_out["k0"][peer_val])
                nc.gpsimd.dma_start(v_sb[:, 16 * D : 24 * D], cc_out["v0"][peer_val])
                nc.gpsimd.dma_start(k_all[:, HALF + SQ : S], cc_out["k1"][peer_val])
                nc.gpsimd.dma_start(v_sb[:, 24 * D : 32 * D], cc_out["v1"][peer_val])

            # ---- attention: software-pipelined blocks ----
            def scores_exp(qb, kp, on_dve):
                q_ap = qT[:, qb * QB : (qb + 1) * QB]
                ps_s = ps_big.tile([128, 2 * QB], F32, tag="ps_big")
                for half in range(2):
                    k = 2 * kp + half
                    nc.tensor.matmul(
                        ps_s[:, half * QB : (half + 1) * QB],
                        lhsT=k_all[:, k * 128 : (k + 1) * 128],
                        rhs=q_ap,
                        start=True,
                        stop=True,
                    )
                ex = exp_pool.tile([128, 2 * QB], BF16, tag="exp")
                if on_dve:
                    nc.vector.tensor_scalar(
                        ex[:].bitcast(mybir.dt.int16),
                        ps_s[:],
                        SCH_A,
                        SCH_B,
                        mybir.AluOpType.mult,
                        mybir.AluOpType.add,
                    )
                else:
                    nc.scalar.activation(ex[:], ps_s[:], AF.Exp, scale=SCALE)
                return ex

            def subtree(exs):
                level = list(exs)
                while len(level) > 1:
                    nxt = []
                    for i in range(0, len(level), 2):
                        if i + 1 < len(level):
                            comb = comb_pool.tile([128, 2 * QB], BF16, tag="comb")
                            nc.vector.tensor_add(comb[:], level[i][:], level[i + 1][:])
                            nxt.append(comb)
                        else:
                            nxt.append(level[i])
                    level = nxt
                return level[0]

            LEAF = 4
            # blocks: (qb, kp_list). Pass X = own keys, pass Y = peer keys;
            # the final block is split in two so the non-overlapped drain
            # (last PV group + ones + evac + store) covers 4 k-pairs, not 8.
            blocks = [(qb, list(range(0, 8))) for qb in range(NQB)]
            blocks += [(qb, list(range(8, 16))) for qb in range(NQB - 1)]
            blocks += [(3, [8, 9, 10, 11]), (3, [12, 13, 14, 15])]

            def emit_stage(cur, prev, extra_pe=None):
                """Interleave cur block's scores+exp with prev block's PV.
                extra_pe: optional per-step PE callables (stage-0 transposes).
                """
                if prev is not None:
                    prev["ps_o"] = ps_acc.tile([128, QB], F32, tag="ps_acc", name="ps_o")
                    prev["ps_sum"] = ps_small.tile(
                        [1, QB], F32, tag="ps_small", name="ps_sum"
                    )
                n_cur = len(cur["kps"]) if cur is not None else 0
                n_prev = len(prev["kps"]) if prev is not None else 0
                for i in range(max(n_cur, n_prev)):
                    if cur is not None and i < n_cur:
                        # DVE-Schraudolph on 2 of 8 exps; with pass-X
                        # evacuations moved to ACT, both exp engines sit
                        # ~1us under the PE stage time (7/1 made ACT the
                        # jitter-limiter, 2/6+DVE-evacs saturated the DVE)
                        cur["exs"].append(
                            scores_exp(cur["qb"], cur["kps"][i], on_dve=(i in (2, 6)))
                        )
                        if (i + 1) % LEAF == 0:
                            cur["roots"].append(
                                subtree(cur["exs"][i + 1 - LEAF : i + 1])
                            )
                    if extra_pe is not None and i < len(extra_pe):
                        extra_pe[i]()
                    if prev is not None and i < n_prev:
                        kp = prev["kps"][i]
                        for half in range(2):
                            k = 2 * kp + half
                            nc.tensor.matmul(
                                prev["ps_o"][:],
                                lhsT=v_sb[:, k * D : (k + 1) * D],
                                rhs=prev["exs"][i][:, half * QB : (half + 1) * QB],
                                start=(i == 0 and half == 0),
                                stop=(i == n_prev - 1 and half == 1),
                            )
                if prev is None:
                    return
                for ri, root in enumerate(prev["roots"]):
                    for half in range(2):
                        nc.tensor.matmul(
                            prev["ps_sum"][:],
                            lhsT=ones_col[:],
                            rhs=root[:, half * QB : (half + 1) * QB],
                            start=(ri == 0 and half == 0),
                            stop=(ri == len(prev["roots"]) - 1 and half == 1),
                        )
                qb = prev["qb"]
                o_sl = o_stage[:, qb * QB : (qb + 1) * QB]
                s_sl = sums_sb[:, qb * QB : (qb + 1) * QB]
                if prev["kps"][0] == 0:  # pass X: stage into SBUF via ACT
                    nc.scalar.activation(o_sl, prev["ps_o"][:], AF.Identity)
                    nc.scalar.activation(s_sl, prev["ps_sum"][:], AF.Identity)
                elif not prev["last"]:  # pass Y, partial: accumulate in place
                    nc.vector.tensor_add(o_sl, o_sl, prev["ps_o"][:])
                    nc.vector.tensor_add(s_sl, s_sl, prev["ps_sum"][:])
                else:  # final contribution for this qb: combine + store
                    o_out = osb_pool.tile([128, QB], F32, tag="osb")
                    nc.vector.tensor_add(o_out[:], o_sl, prev["ps_o"][:])
                    nc.vector.tensor_add(s_sl, s_sl, prev["ps_sum"][:])
                    nc.sync.dma_start(out_d[:, qb * QB : (qb + 1) * QB], o_out[:])
                    nc.sync.dma_start(sums_d[:, qb * QB : (qb + 1) * QB], s_sl)

            prev = None
            for bi, (qb, kps) in enumerate(blocks):
                cur = {
                    "qb": qb,
                    "kps": kps,
                    "exs": [],
                    "roots": [],
                    "last": (kps[-1] == 15),
                }
                extra = None
                if bi == 0:
                    # quarter-1 V transposes ride the first (PV-less) stage
                    extra = [
                        (lambda j=j: transpose_unit(1, j)) for j in range(8)
                    ]
                emit_stage(cur, prev, extra_pe=extra)
                if bi == 0:
                    exchange("v1", v_sb[:, 8 * D : 16 * D])
                    land_peers()
                prev = cur
            emit_stage(None, prev)

    nc.compile()
    return nc


def _prep_inputs(x, W, b):
    """Host-side sharding prep: cast bf16, transpose to xT, slice halves."""
    b_f = np.asarray(b, dtype=np.float32)
    bias_cols = np.ascontiguousarray(b_f.reshape(3, D).T)  # [128, 3]
    w_bf = np.ascontiguousarray(
        np.asarray(W)
        .astype(ml_dtypes.bfloat16)
        .reshape(E // 128, 128, 3 * D)
        .transpose(1, 0, 2)
        .reshape(128, (E // 128) * 3 * D)
    )
    in_maps = []
    for bb in range(B):
        xt_full = np.ascontiguousarray(
            np.asarray(x[bb]).astype(ml_dtypes.bfloat16).T
        )  # [E, S]
        for h in range(2):
            xc = np.ascontiguousarray(xt_full[:, h * HALF : (h + 1) * HALF])
            peer = np.array([[1 - h]], dtype=np.uint32)
            in_maps.append(
                {"xt": xc, "w": w_bf, "bias_cols": bias_cols, "peer": peer}
            )
    return in_maps


def _run(in_maps, trace=False, trace_kwargs=None):
    if "nc" not in _CACHE:
        _CACHE["nc"] = _build()
    return run_bass_kernel_spmd(
        _CACHE["nc"],
        in_maps,
        list(range(N_CORES)),
        trace=trace,
        **(trace_kwargs or {}),
    )


def kernel(x, W, b):
    in_maps = _prep_inputs(x, W, b)
    res = None
    for attempt in range(3):
        try:
            res = _run(in_maps)
            break
        except Exception:
            if attempt == 2:
                raise
    out = np.empty((B, S, D), dtype=np.float32)
    for c in range(N_CORES):
        bb, h = c // 2, c % 2
        o_t = res.results[c]["out_t"]  # [D, HALF]
        sums = res.results[c]["sums"]  # [1, HALF]
        out[bb, h * HALF : (h + 1) * HALF, :] = (o_t / sums).T
    return out


# revision 14
# speedup vs baseline: 1.1299x; 1.0116x over previous
"""Single-head attention (B=4, S=4096, E=2048, d=128) on 8 trn2 cores.

Sharding: core c handles (batch b = c//2, seq half h = c%2). Each core
projects q/k/v for its own 2048-row half; the pair (2b, 2b+1) exchanges
K and V halves via four pairwise AllGathers. Measured CC behavior: the
first mesh cannot begin before ~52us regardless of trigger time (NRT
arming), then meshes run serially at ~7-8us per 256KB — so a dummy
warmup AllGather is fired at ~8us to absorb the arming latency, and the
attention pass is split own/peer (8/8 k-pairs) so peer data is first
consumed ~95us, far behind the worst-case exchange completion (~85us).

Engine/queue plan (all measured):
  sync HWDGE ring: w pieces 0/1, x even e-chunks (both quarters),
    output stores. scalar HWDGE ring: bias, w pieces 2-4, x odd
    e-chunks. Two rings share ~360GB/s; a third (gpsimd SWDGE) path
    starves the scalar ring, so x stays on two rings.
  ACT queue: exp only (plus a tiny warmup activation to preload the
    Exp table before the pass). Projection evacuations run on the DVE
    (tensor_scalar_add with the [128,1] bias column) — ACT evacuations
    behind ring-credit-gated DMA issues measurably slip by >10us.
  gpsimd queue: collective staging DMAs in AND out + triggers. (cc_out
    landings on the sync ring got statically scheduled after pass
    stores, stalling pass-Y PV by ~4us.)

Projection: per streamed x chunk, K, V, Q matmuls (6 x N=512) per
quarter — PE-bound at ~1.28us/chunk vs ~0.72us arrival. V-half
transposes ([d,k]->[k,d] PE identity transposes): quarter-0's ride the
quarter-1 matmul stream, quarter-1's ride the first pass stage.
PSUM: ps_big 3x[128,1024] holds K,V,Q of one quarter, rotating into
the next quarter then score tiles; ps_acc/ps_small hold transposes
during projection, ps_o/ps_sum during the passes.

Attention: blocks of 8 k-pairs x 512 queries, software-pipelined:
block n's score matmuls interleave with block n-1's PV matmuls in the
PE FIFO, so the exp engines (ACT 6 + DVE-Schraudolph 2 per block, ~3%
max rel err, numerically validated to 1/2 of tiles) stay under the
~7.9us PE stage time and score PSUM tiles recycle with slack.
Denominators: DVE pair-add subtrees (leaf=4) at leaf boundaries +
exact ones-column matmuls after the PVs. Output/sums evacuate on the
DVE; host divides and transposes.
"""

import numpy as np
import ml_dtypes

import concourse.tile as tile
from concourse import bacc, mybir
from concourse.bass_utils import run_bass_kernel_spmd
from concourse.masks import make_identity

N_CORES = 8
B, S, E, D = 4, 4096, 2048, 128
HALF = S // 2  # queries / own keys per core
QB = 512  # query block (PSUM bank width in fp32)
SQ = 1024  # projection quarter width
SCALE = 1.0 / float(np.sqrt(D))

BF16 = mybir.dt.bfloat16
F32 = mybir.dt.float32
AF = mybir.ActivationFunctionType

_CACHE = {}


def _build():
    nc = bacc.Bacc(
        trn_type="TRN2", target_bir_lowering=False, debug=False, num_devices=N_CORES
    )

    x_d = nc.dram_tensor("xt", [E, HALF], BF16, kind="ExternalInput").ap()
    w_d = nc.dram_tensor(
        "w", [128, (E // 128) * 3 * D], BF16, kind="ExternalInput"
    ).ap()
    bias_d = nc.dram_tensor("bias_cols", [D, 3], F32, kind="ExternalInput").ap()
    peer_d = nc.dram_tensor("peer", [1, 1], mybir.dt.uint32, kind="ExternalInput").ap()
    out_d = nc.dram_tensor("out_t", [D, HALF], F32, kind="ExternalOutput").ap()
    sums_d = nc.dram_tensor("sums", [1, HALF], F32, kind="ExternalOutput").ap()

    NE = E // 128  # 16 e-chunks
    NQB = HALF // QB  # 4 query blocks
    GROUPS = [[2 * i, 2 * i + 1] for i in range(N_CORES // 2)]

    SCH_A = float(SCALE * (1 << 7) / np.log(2.0))
    SCH_B = float(127 * (1 << 7) + 0.5 - 5.59)

    with tile.TileContext(nc) as tc:
        with (
            tc.tile_pool(name="xt", bufs=32) as xt_pool,
            tc.tile_pool(name="wsb", bufs=1) as w_pool,
            tc.tile_pool(name="persist", bufs=1) as persist,
            tc.tile_pool(name="vtt", bufs=2) as vtt_pool,
            tc.tile_pool(name="exp", bufs=20) as exp_pool,
            tc.tile_pool(name="comb", bufs=8) as comb_pool,
            tc.tile_pool(name="osb", bufs=2) as osb_pool,
            tc.tile_pool(name="dram", bufs=1, space="DRAM") as dram_pool,
            tc.tile_pool(name="ps_big", bufs=3, space="PSUM") as ps_big,
            tc.tile_pool(name="ps_acc", bufs=1, space="PSUM") as ps_acc,
            tc.tile_pool(name="ps_small", bufs=1, space="PSUM") as ps_small,
        ):
            # ---- constants ----
            bias_sb = persist.tile([D, 3], F32, tag="bias")
            nc.scalar.dma_start(bias_sb[:], bias_d[:])
            ones_col = persist.tile([128, 1], BF16, tag="ones")
            nc.gpsimd.memset(ones_col[:], 1.0)
            ident = persist.tile([128, 128], BF16, tag="ident")
            make_identity(nc, ident[:])
            # preload the ACT Exp table now (~1.3us) instead of at the
            # first pass exp
            act_warm = persist.tile([1, 1], BF16, tag="act_warm")
            nc.scalar.activation(act_warm[:], bias_sb[0:1, 0:1], AF.Exp, scale=1.0)

            # ---- CC warmup (see module docstring) ----
            warm_in = dram_pool.tile([1, 2], BF16, tag="warm_in")
            warm_out = dram_pool.tile([2, 1, 2], BF16, tag="warm_out")
            nc.gpsimd.collective_compute(
                "AllGather",
                mybir.AluOpType.bypass,
                replica_groups=GROUPS,
                ins=[warm_in.opt()],
                outs=[warm_out.opt()],
            )

            # ---- w + x loads in consumption order ----
            w_sb = w_pool.tile([128, NE * 3 * D], BF16, tag="w")
            we = 3 * D
            wg = NE * 3 * D // 4  # w quarter piece: 4 e-chunks
            xt = {}
            nc.sync.dma_start(w_sb[:, 0:we], w_d[:, 0:we])
            nc.scalar.dma_start(w_sb[:, wg : 2 * wg], w_d[:, wg : 2 * wg])

            def load_x(sq, e):
                eng = nc.sync if e % 2 == 0 else nc.scalar
                t = xt_pool.tile([128, SQ], BF16, tag="xt", name=f"xt{sq}_{e}")
                eng.dma_start(
                    t[:], x_d[e * 128 : (e + 1) * 128, sq * SQ : (sq + 1) * SQ]
                )
                xt[(sq, e)] = t

            for e in range(NE):
                if e == 2:
                    nc.sync.dma_start(w_sb[:, we:wg], w_d[:, we:wg])
                if e == 4 or e == 5:
                    g = e - 2
                    eng = nc.scalar if e == 4 else nc.sync
                    eng.dma_start(
                        w_sb[:, g * wg : (g + 1) * wg], w_d[:, g * wg : (g + 1) * wg]
                    )
                load_x(0, e)
            for e in range(NE):
                load_x(1, e)

            # peer slot register (host supplies 1 on even cores, 0 on odd).
            # Allocated on gpsimd: the peer landings are gpsimd SWDGE DMAs
            # and register APs are engine-scoped.
            peer_reg = nc.gpsimd.alloc_register("peer_slot")
            nc.gpsimd.reg_load(peer_reg, peer_d[0:1, 0:1])
            peer_val = nc.gpsimd.snap(peer_reg, donate=True, min_val=0, max_val=1)

            qT = persist.tile([D, HALF], BF16, tag="qT")
            k_all = persist.tile([D, S], BF16, tag="k_all")  # [k own | k peer]
            v_sb = persist.tile([128, S // 128 * D], BF16, tag="v")  # own | peer
            sums_sb = persist.tile([1, HALF], F32, tag="sums_sb")
            o_stage = persist.tile([D, HALF], F32, tag="o_stage")

            # ---- collective staging (DRAM) ----
            cc_in = {}
            cc_out = {}
            for nm, shp in (
                ("k0", [D, SQ]),
                ("v0", [128, 8 * D]),
                ("k1", [D, SQ]),
                ("v1", [128, 8 * D]),
            ):
                cc_in[nm] = dram_pool.tile(
                    shp, BF16, tag=f"cc_in_{nm}", name=f"cc_in_{nm}"
                )
                cc_out[nm] = dram_pool.tile(
                    [2] + shp, BF16, tag=f"cc_out_{nm}", name=f"cc_out_{nm}"
                )

            def exchange(nm, src_ap):
                nc.gpsimd.dma_start(cc_in[nm][:], src_ap)
                nc.gpsimd.collective_compute(
                    "AllGather",
                    mybir.AluOpType.bypass,
                    replica_groups=GROUPS,
                    ins=[cc_in[nm].opt()],
                    outs=[cc_out[nm].opt()],
                )

            # ---- projection: per chunk K, V, Q; quarter at a time ----
            vt_tmp = [None, None]

            def transpose_unit(sq, j):
                """One [128,128] PE transpose of vt_tmp[sq] -> v_sb chunk.
                (DMA-transpose was tried: bit-exact but ~7us per 32KB tile
                through the xbar path — useless here.)"""
                pool, ptag = (ps_acc, "ps_acc") if j % 2 == 0 else (ps_small, "ps_small")
                ps_t = pool.tile([128, 128], BF16, tag=ptag)
                nc.tensor.transpose(
                    ps_t[:], vt_tmp[sq][:, j * 128 : (j + 1) * 128], ident[:]
                )
                k = sq * 8 + j
                nc.vector.tensor_copy(v_sb[:, k * D : (k + 1) * D], ps_t[:])

            def dve_evac(dst_ap, ps, g):
                """PSUM -> SBUF with bias add, on the (projection-idle) DVE."""
                nc.vector.tensor_scalar_add(dst_ap, ps[:], bias_sb[:, g : g + 1])

            for sq in range(2):
                ps_k = ps_big.tile([128, SQ], F32, tag="ps_big", name=f"ps_k{sq}")
                ps_v = ps_big.tile([128, SQ], F32, tag="ps_big", name=f"ps_v{sq}")
                ps_q = ps_big.tile([128, SQ], F32, tag="ps_big", name=f"ps_q{sq}")
                for e in range(NE):
                    for g, ps in ((1, ps_k), (2, ps_v), (0, ps_q)):
                        w_ap = w_sb[:, e * 3 * D + g * D : e * 3 * D + (g + 1) * D]
                        for half in range(2):
                            nc.tensor.matmul(
                                ps[:, half * QB : (half + 1) * QB],
                                lhsT=w_ap,
                                rhs=xt[(sq, e)][:, half * QB : (half + 1) * QB],
                                start=(e == 0),
                                stop=(e == NE - 1),
                            )
                    if sq == 1 and e < 8:
                        transpose_unit(0, e)  # quarter-0 V transposes ride here
                dve_evac(k_all[:, sq * SQ : (sq + 1) * SQ], ps_k, 1)
                vt_tmp[sq] = vtt_pool.tile([128, SQ], BF16, tag="vtt", name=f"vtt{sq}")
                dve_evac(vt_tmp[sq][:], ps_v, 2)
                dve_evac(qT[:, sq * SQ : (sq + 1) * SQ], ps_q, 0)
                if sq == 0:
                    exchange("k0", k_all[:, 0:SQ])

            exchange("v0", v_sb[:, 0 : 8 * D])
            exchange("k1", k_all[:, SQ:HALF])
            # v1's staging DMA is emitted after the stage-0 transposes below

            # peer landings, all on the (otherwise idle) gpsimd queue
            def land_peers():
                nc.gpsimd.dma_start(k_all[:, HALF : HALF + SQ], cc_out["k0"][peer_val])
                nc.gpsimd.dma_start(v_sb[:, 16 * D : 24 * D], cc_out["v0"][peer_val])
                nc.gpsimd.dma_start(k_all[:, HALF + SQ : S], cc_out["k1"][peer_val])
                nc.gpsimd.dma_start(v_sb[:, 24 * D : 32 * D], cc_out["v1"][peer_val])

            # ---- attention: software-pipelined blocks ----
            def scores_exp(qb, kp, on_dve):
                q_ap = qT[:, qb * QB : (qb + 1) * QB]
                ps_s = ps_big.tile([128, 2 * QB], F32, tag="ps_big")
                for half in range(2):
                    k = 2 * kp + half
                    nc.tensor.matmul(
                        ps_s[:, half * QB : (half + 1) * QB],
                        lhsT=k_all[:, k * 128 : (k + 1) * 128],
                        rhs=q_ap,
                        start=True,
                        stop=True,
                    )
                ex = exp_pool.tile([128, 2 * QB], BF16, tag="exp")
                if on_dve:
                    nc.vector.tensor_scalar(
                        ex[:].bitcast(mybir.dt.int16),
                        ps_s[:],
                        SCH_A,
                        SCH_B,
                        mybir.AluOpType.mult,
                        mybir.AluOpType.add,
                    )
                else:
                    nc.scalar.activation(ex[:], ps_s[:], AF.Exp, scale=SCALE)
                return ex

            def subtree(exs):
                level = list(exs)
                while len(level) > 1:
                    nxt = []
                    for i in range(0, len(level), 2):
                        if i + 1 < len(level):
                            comb = comb_pool.tile([128, 2 * QB], BF16, tag="comb")
                            nc.vector.tensor_add(comb[:], level[i][:], level[i + 1][:])
                            nxt.append(comb)
                        else:
                            nxt.append(level[i])
                    level = nxt
                return level[0]

            LEAF = 4
            # blocks: (qb, kp_list). Pass X = own keys, pass Y = peer keys;
            # the final block is split in two so the non-overlapped drain
            # (last PV group + ones + evac + store) covers 4 k-pairs, not 8.
            blocks = [(qb, list(range(0, 8))) for qb in range(NQB)]
            blocks += [(qb, list(range(8, 16))) for qb in range(NQB - 1)]
            blocks += [(3, [8, 9, 10, 11]), (3, [12, 13, 14, 15])]

            def emit_stage(cur, prev, extra_pe=None):
                """Interleave cur block's scores+exp with prev block's PV.
                extra_pe: optional per-step PE callables (stage-0 transposes).
                """
                if prev is not None:
                    prev["ps_o"] = ps_acc.tile([128, QB], F32, tag="ps_acc", name="ps_o")
                    prev["ps_sum"] = ps_small.tile(
                        [1, QB], F32, tag="ps_small", name="ps_sum"
                    )
                leaf = (
                    8 if (cur is not None and cur["kps"][0] == 0) else LEAF
                )  # X: 1 root/2 ones-MMs, DVE has slack for the extra add
                n_cur = len(cur["kps"]) if cur is not None else 0
                n_prev = len(prev["kps"]) if prev is not None else 0
                for i in range(max(n_cur, n_prev)):
                    if cur is not None and i < n_cur:
                        # DVE-Schraudolph on 2 of 8 exps; with pass-X
                        # evacuations moved to ACT, both exp engines sit
                        # ~1us under the PE stage time (7/1 made ACT the
                        # jitter-limiter, 2/6+DVE-evacs saturated the DVE)
                        cur["exs"].append(
                            scores_exp(cur["qb"], cur["kps"][i], on_dve=(i in (2, 6)))
                        )
                        if (i + 1) % leaf == 0:
                            cur["roots"].append(
                                subtree(cur["exs"][i + 1 - leaf : i + 1])
                            )
                    if extra_pe is not None and i < len(extra_pe):
                        extra_pe[i]()
                    if prev is not None and i < n_prev:
                        kp = prev["kps"][i]
                        for half in range(2):
                            k = 2 * kp + half
                            nc.tensor.matmul(
                                prev["ps_o"][:],
                                lhsT=v_sb[:, k * D : (k + 1) * D],
                                rhs=prev["exs"][i][:, half * QB : (half + 1) * QB],
                                start=(i == 0 and half == 0),
                                stop=(i == n_prev - 1 and half == 1),
                            )
                if prev is None:
                    return
                for ri, root in enumerate(prev["roots"]):
                    for half in range(2):
                        nc.tensor.matmul(
                            prev["ps_sum"][:],
                            lhsT=ones_col[:],
                            rhs=root[:, half * QB : (half + 1) * QB],
                            start=(ri == 0 and half == 0),
                            stop=(ri == len(prev["roots"]) - 1 and half == 1),
                        )
                qb = prev["qb"]
                o_sl = o_stage[:, qb * QB : (qb + 1) * QB]
                s_sl = sums_sb[:, qb * QB : (qb + 1) * QB]
                if prev["kps"][0] == 0:  # pass X: stage into SBUF via ACT
                    nc.scalar.activation(o_sl, prev["ps_o"][:], AF.Identity)
                    nc.scalar.activation(s_sl, prev["ps_sum"][:], AF.Identity)
                elif not prev["last"]:  # pass Y, partial: accumulate in place
                    nc.vector.tensor_add(o_sl, o_sl, prev["ps_o"][:])
                    nc.vector.tensor_add(s_sl, s_sl, prev["ps_sum"][:])
                else:  # final contribution for this qb: combine + store
                    o_out = osb_pool.tile([128, QB], F32, tag="osb")
                    nc.vector.tensor_add(o_out[:], o_sl, prev["ps_o"][:])
                    nc.vector.tensor_add(s_sl, s_sl, prev["ps_sum"][:])
                    nc.sync.dma_start(out_d[:, qb * QB : (qb + 1) * QB], o_out[:])
                    nc.sync.dma_start(sums_d[:, qb * QB : (qb + 1) * QB], s_sl)

            prev = None
            for bi, (qb, kps) in enumerate(blocks):
                cur = {
                    "qb": qb,
                    "kps": kps,
                    "exs": [],
                    "roots": [],
                    "last": (kps[-1] == 15),
                }
                extra = None
                if bi == 0:
                    # quarter-1 V transposes ride the first (PV-less) stage
                    extra = [
                        (lambda j=j: transpose_unit(1, j)) for j in range(8)
                    ]
                emit_stage(cur, prev, extra_pe=extra)
                if bi == 0:
                    exchange("v1", v_sb[:, 8 * D : 16 * D])
                    land_peers()
                prev = cur
            emit_stage(None, prev)

    nc.compile()
    return nc


def _prep_inputs(x, W, b):
    """Host-side sharding prep: cast bf16, transpose to xT, slice halves."""
    b_f = np.asarray(b, dtype=np.float32)
    bias_cols = np.ascontiguousarray(b_f.reshape(3, D).T)  # [128, 3]
    w_bf = np.ascontiguousarray(
        np.asarray(W)
        .astype(ml_dtypes.bfloat16)
        .reshape(E // 128, 128, 3 * D)
        .transpose(1, 0, 2)
        .reshape(128, (E // 128) * 3 * D)
    )
    in_maps = []
    for bb in range(B):
        xt_full = np.ascontiguousarray(
            np.asarray(x[bb]).astype(ml_dtypes.bfloat16).T
        )  # [E, S]
        for h in range(2):
            xc = np.ascontiguousarray(xt_full[:, h * HALF : (h + 1) * HALF])
            peer = np.array([[1 - h]], dtype=np.uint32)
            in_maps.append(
                {"xt": xc, "w": w_bf, "bias_cols": bias_cols, "peer": peer}
            )
    return in_maps


def _run(in_maps, trace=False, trace_kwargs=None):
    if "nc" not in _CACHE:
        _CACHE["nc"] = _build()
    return run_bass_kernel_spmd(
        _CACHE["nc"],
        in_maps,
        list(range(N_CORES)),
        trace=trace,
        **(trace_kwargs or {}),
    )


def kernel(x, W, b):
    in_maps = _prep_inputs(x, W, b)
    res = None
    for attempt in range(3):
        try:
            res = _run(in_maps)
            break
        except Exception:
            if attempt == 2:
                raise
    out = np.empty((B, S, D), dtype=np.float32)
    for c in range(N_CORES):
        bb, h = c // 2, c % 2
        o_t = res.results[c]["out_t"]  # [D, HALF]
        sums = res.results[c]["sums"]  # [1, HALF]
        out[bb, h * HALF : (h + 1) * HALF, :] = (o_t / sums).T
    return out


# revision 16
# speedup vs baseline: 1.1318x; 1.0017x over previous
"""Single-head attention (B=4, S=4096, E=2048, d=128) on 8 trn2 cores.

Sharding: core c handles (batch b = c//2, seq half h = c%2). Each core
projects q/k/v for its own 2048-row half; the pair (2b, 2b+1) exchanges
K and V halves via four pairwise AllGathers. Measured CC behavior: the
first mesh cannot begin before ~52us regardless of trigger time (NRT
arming), then meshes run serially at ~7-8us per 256KB — so a dummy
warmup AllGather is fired at ~8us to absorb the arming latency, and the
attention pass is split own/peer (8/8 k-pairs) so peer data is first
consumed ~95us, far behind the worst-case exchange completion (~85us).

Engine/queue plan (all measured):
  sync HWDGE ring: w pieces 0/1, x even e-chunks (both quarters),
    output stores. scalar HWDGE ring: bias, w pieces 2-4, x odd
    e-chunks. Two rings share ~360GB/s; a third (gpsimd SWDGE) path
    starves the scalar ring, so x stays on two rings.
  ACT queue: exp only (plus a tiny warmup activation to preload the
    Exp table before the pass). Projection evacuations run on the DVE
    (tensor_scalar_add with the [128,1] bias column) — ACT evacuations
    behind ring-credit-gated DMA issues measurably slip by >10us.
  gpsimd queue: collective staging DMAs in AND out + triggers. (cc_out
    landings on the sync ring got statically scheduled after pass
    stores, stalling pass-Y PV by ~4us.)

Projection: per streamed x chunk, K, V, Q matmuls (6 x N=512) per
quarter — PE-bound at ~1.28us/chunk vs ~0.72us arrival. V-half
transposes ([d,k]->[k,d] PE identity transposes): quarter-0's ride the
quarter-1 matmul stream, quarter-1's ride the first pass stage.
PSUM: ps_big 3x[128,1024] holds K,V,Q of one quarter, rotating into
the next quarter then score tiles; ps_acc/ps_small hold transposes
during projection, ps_o/ps_sum during the passes.

Attention: blocks of 8 k-pairs x 512 queries, software-pipelined:
block n's score matmuls interleave with block n-1's PV matmuls in the
PE FIFO, so the exp engines (ACT 6 + DVE-Schraudolph 2 per block, ~3%
max rel err, numerically validated to 1/2 of tiles) stay under the
~7.9us PE stage time and score PSUM tiles recycle with slack.
Denominators: DVE pair-add subtrees (leaf=4) at leaf boundaries +
exact ones-column matmuls after the PVs. Output/sums evacuate on the
DVE; host divides and transposes.
"""

import numpy as np
import ml_dtypes

import concourse.tile as tile
from concourse import bacc, mybir
from concourse.bass_utils import run_bass_kernel_spmd
from concourse.masks import make_identity

N_CORES = 8
B, S, E, D = 4, 4096, 2048, 128
HALF = S // 2  # queries / own keys per core
QB = 512  # query block (PSUM bank width in fp32)
SQ = 1024  # projection quarter width
SCALE = 1.0 / float(np.sqrt(D))

BF16 = mybir.dt.bfloat16
F32 = mybir.dt.float32
AF = mybir.ActivationFunctionType

_CACHE = {}


def _build():
    nc = bacc.Bacc(
        trn_type="TRN2", target_bir_lowering=False, debug=False, num_devices=N_CORES
    )

    x_d = nc.dram_tensor("xt", [E, HALF], BF16, kind="ExternalInput").ap()
    w_d = nc.dram_tensor(
        "w", [128, (E // 128) * 3 * D], BF16, kind="ExternalInput"
    ).ap()
    bias_d = nc.dram_tensor("bias_cols", [D, 3], F32, kind="ExternalInput").ap()
    peer_d = nc.dram_tensor("peer", [1, 1], mybir.dt.uint32, kind="ExternalInput").ap()
    out_d = nc.dram_tensor("out_t", [D, HALF], F32, kind="ExternalOutput").ap()
    sums_d = nc.dram_tensor("sums", [1, HALF], F32, kind="ExternalOutput").ap()

    NE = E // 128  # 16 e-chunks
    NQB = HALF // QB  # 4 query blocks
    GROUPS = [[2 * i, 2 * i + 1] for i in range(N_CORES // 2)]

    SCH_A = float(SCALE * (1 << 7) / np.log(2.0))
    SCH_B = float(127 * (1 << 7) + 0.5 - 5.59)

    with tile.TileContext(nc) as tc:
        with (
            tc.tile_pool(name="xt", bufs=32) as xt_pool,
            tc.tile_pool(name="wsb", bufs=1) as w_pool,
            tc.tile_pool(name="persist", bufs=1) as persist,
            tc.tile_pool(name="vtt", bufs=2) as vtt_pool,
            tc.tile_pool(name="exp", bufs=20) as exp_pool,
            tc.tile_pool(name="comb", bufs=8) as comb_pool,
            tc.tile_pool(name="osb", bufs=2) as osb_pool,
            tc.tile_pool(name="dram", bufs=1, space="DRAM") as dram_pool,
            tc.tile_pool(name="ps_big", bufs=3, space="PSUM") as ps_big,
            tc.tile_pool(name="ps_acc", bufs=1, space="PSUM") as ps_acc,
            tc.tile_pool(name="ps_small", bufs=1, space="PSUM") as ps_small,
        ):
            # ---- constants ----
            bias_sb = persist.tile([D, 3], F32, tag="bias")
            nc.scalar.dma_start(bias_sb[:], bias_d[:])
            ones_col = persist.tile([128, 1], BF16, tag="ones")
            nc.gpsimd.memset(ones_col[:], 1.0)
            ident = persist.tile([128, 128], BF16, tag="ident")
            make_identity(nc, ident[:])
            # preload the ACT Exp table now (~1.3us) instead of at the
            # first pass exp
            act_warm = persist.tile([1, 1], BF16, tag="act_warm")
            nc.scalar.activation(act_warm[:], bias_sb[0:1, 0:1], AF.Exp, scale=1.0)

            # ---- CC warmup (see module docstring) ----
            warm_in = dram_pool.tile([1, 2], BF16, tag="warm_in")
            warm_out = dram_pool.tile([2, 1, 2], BF16, tag="warm_out")
            nc.gpsimd.collective_compute(
                "AllGather",
                mybir.AluOpType.bypass,
                replica_groups=GROUPS,
                ins=[warm_in.opt()],
                outs=[warm_out.opt()],
            )

            # ---- w + x loads in consumption order ----
            w_sb = w_pool.tile([128, NE * 3 * D], BF16, tag="w")
            we = 3 * D
            wg = NE * 3 * D // 4  # w quarter piece: 4 e-chunks
            xt = {}
            nc.sync.dma_start(w_sb[:, 0:we], w_d[:, 0:we])
            nc.scalar.dma_start(w_sb[:, wg : 2 * wg], w_d[:, wg : 2 * wg])

            def load_x(sq, e):
                eng = nc.sync if e % 2 == 0 else nc.scalar
                t = xt_pool.tile([128, SQ], BF16, tag="xt", name=f"xt{sq}_{e}")
                eng.dma_start(
                    t[:], x_d[e * 128 : (e + 1) * 128, sq * SQ : (sq + 1) * SQ]
                )
                xt[(sq, e)] = t

            for e in range(NE):
                if e == 2:
                    nc.sync.dma_start(w_sb[:, we:wg], w_d[:, we:wg])
                if e == 4 or e == 5:
                    g = e - 2
                    eng = nc.scalar if e == 4 else nc.sync
                    eng.dma_start(
                        w_sb[:, g * wg : (g + 1) * wg], w_d[:, g * wg : (g + 1) * wg]
                    )
                load_x(0, e)
            for e in range(NE):
                load_x(1, e)

            # peer slot register (host supplies 1 on even cores, 0 on odd).
            # Allocated on gpsimd: the peer landings are gpsimd SWDGE DMAs
            # and register APs are engine-scoped.
            peer_reg = nc.gpsimd.alloc_register("peer_slot")
            nc.gpsimd.reg_load(peer_reg, peer_d[0:1, 0:1])
            peer_val = nc.gpsimd.snap(peer_reg, donate=True, min_val=0, max_val=1)

            qT = persist.tile([D, HALF], BF16, tag="qT")
            k_all = persist.tile([D, S], BF16, tag="k_all")  # [k own | k peer]
            v_sb = persist.tile([128, S // 128 * D], BF16, tag="v")  # own | peer
            sums_sb = persist.tile([1, HALF], F32, tag="sums_sb")
            o_stage = persist.tile([D, HALF], F32, tag="o_stage")

            # ---- collective staging (DRAM) ----
            cc_in = {}
            cc_out = {}
            for nm, shp in (
                ("k0", [D, SQ]),
                ("v0", [128, 8 * D]),
                ("k1", [D, SQ]),
                ("v1", [128, 8 * D]),
            ):
                cc_in[nm] = dram_pool.tile(
                    shp, BF16, tag=f"cc_in_{nm}", name=f"cc_in_{nm}"
                )
                cc_out[nm] = dram_pool.tile(
                    [2] + shp, BF16, tag=f"cc_out_{nm}", name=f"cc_out_{nm}"
                )

            def exchange(nm, src_ap):
                nc.gpsimd.dma_start(cc_in[nm][:], src_ap)
                nc.gpsimd.collective_compute(
                    "AllGather",
                    mybir.AluOpType.bypass,
                    replica_groups=GROUPS,
                    ins=[cc_in[nm].opt()],
                    outs=[cc_out[nm].opt()],
                )

            # ---- projection: per chunk K, V, Q; quarter at a time ----
            vt_tmp = [None, None]

            def transpose_unit(sq, j):
                """One [128,128] PE transpose of vt_tmp[sq] -> v_sb chunk.
                (DMA-transpose was tried: bit-exact but ~7us per 32KB tile
                through the xbar path — useless here.)"""
                pool, ptag = (ps_acc, "ps_acc") if j % 2 == 0 else (ps_small, "ps_small")
                ps_t = pool.tile([128, 128], BF16, tag=ptag)
                nc.tensor.transpose(
                    ps_t[:], vt_tmp[sq][:, j * 128 : (j + 1) * 128], ident[:]
                )
                k = sq * 8 + j
                nc.vector.tensor_copy(v_sb[:, k * D : (k + 1) * D], ps_t[:])

            def dve_evac(dst_ap, ps, g):
                """PSUM -> SBUF with bias add, on the (projection-idle) DVE."""
                nc.vector.tensor_scalar_add(dst_ap, ps[:], bias_sb[:, g : g + 1])

            for sq in range(2):
                ps_k = ps_big.tile([128, SQ], F32, tag="ps_big", name=f"ps_k{sq}")
                ps_v = ps_big.tile([128, SQ], F32, tag="ps_big", name=f"ps_v{sq}")
                ps_q = ps_big.tile([128, SQ], F32, tag="ps_big", name=f"ps_q{sq}")
                for e in range(NE):
                    for g, ps in ((1, ps_k), (2, ps_v), (0, ps_q)):
                        w_ap = w_sb[:, e * 3 * D + g * D : e * 3 * D + (g + 1) * D]
                        for half in range(2):
                            nc.tensor.matmul(
                                ps[:, half * QB : (half + 1) * QB],
                                lhsT=w_ap,
                                rhs=xt[(sq, e)][:, half * QB : (half + 1) * QB],
                                start=(e == 0),
                                stop=(e == NE - 1),
                            )
                    if sq == 1 and e < 8:
                        transpose_unit(0, e)  # quarter-0 V transposes ride here
                dve_evac(k_all[:, sq * SQ : (sq + 1) * SQ], ps_k, 1)
                vt_tmp[sq] = vtt_pool.tile([128, SQ], BF16, tag="vtt", name=f"vtt{sq}")
                dve_evac(vt_tmp[sq][:], ps_v, 2)
                dve_evac(qT[:, sq * SQ : (sq + 1) * SQ], ps_q, 0)
                if sq == 0:
                    exchange("k0", k_all[:, 0:SQ])

            exchange("v0", v_sb[:, 0 : 8 * D])
            exchange("k1", k_all[:, SQ:HALF])
            # v1's staging DMA is emitted after the stage-0 transposes below

            # peer landings, all on the (otherwise idle) gpsimd queue
            def land_peers():
                nc.gpsimd.dma_start(k_all[:, HALF : HALF + SQ], cc_out["k0"][peer_val])
                nc.gpsimd.dma_start(v_sb[:, 16 * D : 24 * D], cc_out["v0"][peer_val])
                nc.gpsimd.dma_start(k_all[:, HALF + SQ : S], cc_out["k1"][peer_val])
                nc.gpsimd.dma_start(v_sb[:, 24 * D : 32 * D], cc_out["v1"][peer_val])

            # ---- attention: software-pipelined blocks ----
            def scores_exp(qb, kp, on_dve):
                q_ap = qT[:, qb * QB : (qb + 1) * QB]
                ps_s = ps_big.tile([128, 2 * QB], F32, tag="ps_big")
                for half in range(2):
                    k = 2 * kp + half
                    nc.tensor.matmul(
                        ps_s[:, half * QB : (half + 1) * QB],
                        lhsT=k_all[:, k * 128 : (k + 1) * 128],
                        rhs=q_ap,
                        start=True,
                        stop=True,
                    )
                ex = exp_pool.tile([128, 2 * QB], BF16, tag="exp")
                if on_dve:
                    nc.vector.tensor_scalar(
                        ex[:].bitcast(mybir.dt.int16),
                        ps_s[:],
                        SCH_A,
                        SCH_B,
                        mybir.AluOpType.mult,
                        mybir.AluOpType.add,
                    )
                else:
                    nc.scalar.activation(ex[:], ps_s[:], AF.Exp, scale=SCALE)
                return ex

            def subtree(exs):
                level = list(exs)
                while len(level) > 1:
                    nxt = []
                    for i in range(0, len(level), 2):
                        if i + 1 < len(level):
                            comb = comb_pool.tile([128, 2 * QB], BF16, tag="comb")
                            nc.vector.tensor_add(comb[:], level[i][:], level[i + 1][:])
                            nxt.append(comb)
                        else:
                            nxt.append(level[i])
                    level = nxt
                return level[0]

            LEAF = 4
            # blocks: (qb, kp_list). Pass X = own keys, pass Y = peer keys;
            # the final block is split in two so the non-overlapped drain
            # (last PV group + ones + evac + store) covers 4 k-pairs, not 8.
            blocks = [(qb, list(range(0, 8))) for qb in range(NQB)]
            blocks += [(qb, list(range(8, 16))) for qb in range(NQB - 1)]
            blocks += [(3, [8, 9, 10, 11]), (3, [12, 13, 14, 15])]

            def emit_stage(cur, prev, extra_pe=None):
                """Interleave cur block's scores+exp with prev block's PV.
                extra_pe: optional per-step PE callables (stage-0 transposes).
                """
                if prev is not None:
                    prev["ps_o"] = ps_acc.tile([128, QB], F32, tag="ps_acc", name="ps_o")
                    prev["ps_sum"] = ps_small.tile(
                        [1, QB], F32, tag="ps_small", name="ps_sum"
                    )
                leaf = (
                    8 if (cur is not None and cur["kps"][0] == 0) else LEAF
                )  # X: 1 root/2 ones-MMs, DVE has slack for the extra add
                n_cur = len(cur["kps"]) if cur is not None else 0
                n_prev = len(prev["kps"]) if prev is not None else 0
                for i in range(max(n_cur, n_prev)):
                    if cur is not None and i < n_cur:
                        # DVE-Schraudolph on 2 of 8 exps; with pass-X
                        # evacuations moved to ACT, both exp engines sit
                        # ~1us under the PE stage time (7/1 made ACT the
                        # jitter-limiter, 2/6+DVE-evacs saturated the DVE)
                        cur["exs"].append(
                            scores_exp(cur["qb"], cur["kps"][i], on_dve=(i in (2, 6)))
                        )
                        if (i + 1) % leaf == 0:
                            cur["roots"].append(
                                subtree(cur["exs"][i + 1 - leaf : i + 1])
                            )
                    if extra_pe is not None and i < len(extra_pe):
                        extra_pe[i]()
                    if prev is not None and i < n_prev:
                        kp = prev["kps"][i]
                        for half in range(2):
                            k = 2 * kp + half
                            nc.tensor.matmul(
                                prev["ps_o"][:],
                                lhsT=v_sb[:, k * D : (k + 1) * D],
                                rhs=prev["exs"][i][:, half * QB : (half + 1) * QB],
                                start=(i == 0 and half == 0),
                                stop=(i == n_prev - 1 and half == 1),
                            )
                if prev is None:
                    return
                for ri, root in enumerate(prev["roots"]):
                    for half in range(2):
                        nc.tensor.matmul(
                            prev["ps_sum"][:],
                            lhsT=ones_col[:],
                            rhs=root[:, half * QB : (half + 1) * QB],
                            start=(ri == 0 and half == 0),
                            stop=(ri == len(prev["roots"]) - 1 and half == 1),
                        )
                qb = prev["qb"]
                o_sl = o_stage[:, qb * QB : (qb + 1) * QB]
                s_sl = sums_sb[:, qb * QB : (qb + 1) * QB]
                if prev["kps"][0] == 0:  # pass X: stage into SBUF via ACT
                    nc.scalar.activation(o_sl, prev["ps_o"][:], AF.Identity)
                    nc.scalar.activation(s_sl, prev["ps_sum"][:], AF.Identity)
                elif not prev["last"]:  # pass Y, partial: accumulate in place
                    nc.vector.tensor_add(o_sl, o_sl, prev["ps_o"][:])
                    nc.vector.tensor_add(s_sl, s_sl, prev["ps_sum"][:])
                else:  # final contribution for this qb: combine + store
                    o_out = osb_pool.tile([128, QB], F32, tag="osb")
                    nc.vector.tensor_add(o_out[:], o_sl, prev["ps_o"][:])
                    nc.vector.tensor_add(s_sl, s_sl, prev["ps_sum"][:])
                    nc.sync.dma_start(out_d[:, qb * QB : (qb + 1) * QB], o_out[:])
                    nc.sync.dma_start(sums_d[:, qb * QB : (qb + 1) * QB], s_sl)

            prev = None
            for bi, (qb, kps) in enumerate(blocks):
                cur = {
                    "qb": qb,
                    "kps": kps,
                    "exs": [],
                    "roots": [],
                    "last": (kps[-1] == 15),
                }
                extra = None
                if bi == 0:
                    # quarter-1 V transposes ride the first (PV-less) stage
                    extra = [
                        (lambda j=j: transpose_unit(1, j)) for j in range(8)
                    ]
                emit_stage(cur, prev, extra_pe=extra)
                if bi == 0:
                    exchange("v1", v_sb[:, 8 * D : 16 * D])
                    land_peers()
                prev = cur
            emit_stage(None, prev)

    nc.compile()
    return nc


def _prep_inputs(x, W, b):
    """Host-side sharding prep: cast bf16, transpose to xT, slice halves."""
    b_f = np.asarray(b, dtype=np.float32)
    bias_cols = np.ascontiguousarray(b_f.reshape(3, D).T)  # [128, 3]
    w_bf = np.ascontiguousarray(
        np.asarray(W)
        .astype(ml_dtypes.bfloat16)
        .reshape(E // 128, 128, 3 * D)
        .transpose(1, 0, 2)
        .reshape(128, (E // 128) * 3 * D)
    )
    in_maps = []
    for bb in range(B):
        xt_full = np.ascontiguousarray(
            np.asarray(x[bb]).astype(ml_dtypes.bfloat16).T
        )  # [E, S]
        for h in range(2):
            xc = np.ascontiguousarray(xt_full[:, h * HALF : (h + 1) * HALF])
            peer = np.array([[1 - h]], dtype=np.uint32)
            in_maps.append(
                {"xt": xc, "w": w_bf, "bias_cols": bias_cols, "peer": peer}
            )
    return in_maps


def _run(in_maps, trace=False, trace_kwargs=None):
    if "nc" not in _CACHE:
        _CACHE["nc"] = _build()
    return run_bass_kernel_spmd(
        _CACHE["nc"],
        in_maps,
        list(range(N_CORES)),
        trace=trace,
        **(trace_kwargs or {}),
    )


def kernel(x, W, b):
    in_maps = _prep_inputs(x, W, b)
    res = None
    for attempt in range(3):
        try:
            res = _run(in_maps)
            break
        except Exception:
            if attempt == 2:
                raise
    out = np.empty((B, S, D), dtype=np.float32)
    for c in range(N_CORES):
        bb, h = c // 2, c % 2
        o_t = res.results[c]["out_t"]  # [D, HALF]
        sums = res.results[c]["sums"]  # [1, HALF]
        out[bb, h * HALF : (h + 1) * HALF, :] = (o_t / sums).T
    return out
